# revision 18
# baseline (speedup 1.0000x reference)
"""Trainium2 Bass kernel for the GNN k-hop subgraph encoder (GIN, L=2, D=256).

Strategy (8 cores, graph-parallel), v2:
  - Host: sort subgraph nodes by indicator (center id); shard at center
    boundaries (2500 centers/core); slotted per-core layout so every
    128-center block owns a fixed number of 128-row tiles.
  - Layer 1 needs NO gather: node/edge embedding types have tiny
    cardinality, so layer-1 aggregation is counts^T @ table.
  - h1 is published via a Shared-output AllGather (one shared HBM buffer
    for all 8 cores instead of 8 replicated 63MB copies).
  - Layer 2 gathers h1[src] rows with ONE batched indirect DMA per
    512-slot chunk (SWDGE fixed overhead ~1us/instruction dominates, so
    batching ~20 tiles per instruction is ~10x cheaper than per-tile).
  - Scatter-add one-hot matrices are generated ON DEVICE from int32 dst
    columns (iota + is_equal), killing ~21MB/core of one-hot DMA traffic.
  - Pooling onto centers is interleaved with layer 2 (block pooled as
    soon as its 6 h2 tiles exist), also via on-device one-hots.
  - BatchNorm stats (2x) via tiny AllReduce; projection + final norm +
    transpose on device. Host concatenates the 8 output slices.
All matmul operands fp16 (PE 1 cycle/row), accumulation fp32 in PSUM.
"""
import os
import sys

import numpy as np

sys.path.insert(0, "/opt/trn_rl_repo")

N = 20000
NSUB = 100000
ESUB = 300000
D = 256
EPS = 1e-5
NCORE = 8
CPC = N // NCORE            # 2500 centers per core
CPAD = 2560                 # padded to 20 blocks of 128
NBLK = CPAD // 128          # 20


def _rowperm(slot):
    """DRAM row of local slot s in h1loc: layer-1 chunk stores are flat
    [128, 1024] SBUF->DRAM copies, so slot (ch, rt, p) lands at row
    ch*512 + p*4 + rt."""
    ch = slot // 512
    r = slot % 512
    rt = r // 128
    p = r % 128
    return ch * 512 + p * 4 + rt


# ----------------------------------------------------------------------------
# host preprocessing
# ----------------------------------------------------------------------------
def _preprocess(inputs):
    x = np.asarray(inputs["x"], np.int64)
    sni = np.asarray(inputs["subgraph_node_index"], np.int64)
    sei = np.asarray(inputs["subgraph_edge_index"], np.int64)
    sea = np.asarray(inputs["subgraph_edge_attr"], np.int64)
    sii = np.asarray(inputs["subgraph_indicator_index"], np.int64)

    pi = np.argsort(sii, kind="stable")
    inv = np.empty(NSUB, np.int64)
    inv[pi] = np.arange(NSUB)
    ind_s = sii[pi]
    node_s = sni[pi]

    src = inv[sei[0]]
    dst = inv[sei[1]]
    sl = np.arange(NSUB)
    src = np.concatenate([src, sl])
    dst = np.concatenate([dst, sl])
    ea0 = np.concatenate([sea[:, 0], np.full(NSUB, 4, np.int64)])
    ea1 = np.concatenate([sea[:, 1], np.zeros(NSUB, np.int64)])
    ecombo = np.where(ea0 == 4, 9, ea0 * 3 + ea1)
    ntype = x[node_s, 0] * 3 + x[node_s, 1]

    sub_lo = np.searchsorted(ind_s, np.arange(0, N + 1, CPC))
    core_of_pos = np.searchsorted(sub_lo, np.arange(NSUB), side="right") - 1
    blk_of_pos = (ind_s - core_of_pos * CPC) // 128
    cnt_cb = np.zeros((NCORE, NBLK), np.int64)
    np.add.at(cnt_cb, (core_of_pos, blk_of_pos), 1)
    S_max = int(np.ceil(cnt_cb.max() / 128))
    SLOTS = NBLK * S_max * 128

    # slot of each subgraph position within its (core, blk): balance the
    # per-128-slot-tile edge counts (LPT on in-degree) so tiles_per_nt --
    # and hence the number of gather instructions -- is minimized
    deg = np.bincount(dst[:ESUB], minlength=NSUB)
    slot = np.zeros(NSUB, np.int64)
    for c in range(NCORE):
        for b in range(NBLK):
            m = np.where((core_of_pos == c) & (blk_of_pos == b))[0]
            o = m[np.argsort(-deg[m], kind="stable")]
            loads = np.zeros(S_max, np.int64)
            fill = np.zeros(S_max, np.int64)
            base = b * S_max * 128
            for i in o:
                cand = np.flatnonzero(fill < 128)
                tsel = cand[np.argmin(loads[cand])]
                slot[i] = base + tsel * 128 + fill[tsel]
                loads[tsel] += deg[i]
                fill[tsel] += 1
    # permuted DRAM row of each subgraph position's h1 entry (piece-major
    # layout: h1full = [piece][core][PIECE rows])
    _q = _rowperm(slot)

    dst_core = core_of_pos[dst]
    ntile = SLOTS // 128
    dst_slot = slot[dst]
    # real (non-self-loop) edges only: self-loop h1 contribution is added on
    # device via identity matmuls from the core's own h1 tiles
    real = np.zeros(len(src), bool)
    real[:ESUB] = True
    dst_tile = dst_slot // 128
    e_cnt = np.zeros((NCORE, ntile), np.int64)
    np.add.at(e_cnt, (dst_core[real], dst_tile[real]), 1)
    tiles_per_nt = np.ceil(e_cnt.max(axis=0) / 128).astype(np.int64)
    T_E = int(tiles_per_nt.sum())
    NPIECE = 1
    PIECE = SLOTS // NPIECE

    gslotp = ((_q // PIECE) * NCORE * PIECE + core_of_pos * PIECE
              + (_q % PIECE))

    per_core = []
    for c in range(NCORE):
        em = (dst_core == c) & real
        ema = dst_core == c   # all edges incl self-loops (for counts)
        cnt19 = np.zeros((19, SLOTS), np.float16)
        np.add.at(cnt19, (ntype[src[ema]], dst_slot[ema]), 1.0)
        np.add.at(cnt19, (9 + ecombo[ema], dst_slot[ema]), 1.0)

        gidxT = np.zeros((128, T_E), np.float32)
        dstT = np.full((128, T_E), -1, np.float32)
        es_all, ed_all = src[em], dst_slot[em]
        o = np.argsort(ed_all, kind="stable")
        es_all, ed_all = es_all[o], ed_all[o]
        ed_tile = ed_all // 128
        bounds = np.searchsorted(ed_tile, np.arange(ntile + 1))
        t0 = 0
        for nt in range(ntile):
            a, b = bounds[nt], bounds[nt + 1]
            es, ed = es_all[a:b], ed_all[a:b]
            k = b - a
            for t in range(int(tiles_per_nt[nt])):
                lo2, hi2 = t * 128, min((t + 1) * 128, k)
                if hi2 > lo2:
                    m = hi2 - lo2
                    gidxT[:m, t0] = gslotp[es[lo2:hi2]]
                    dstT[:m, t0] = ed[lo2:hi2] - nt * 128
                t0 += 1
        assert t0 == T_E

        lo, hi = sub_lo[c], sub_lo[c + 1]
        ind_l = np.full(SLOTS, -1, np.int64)
        ind_l[slot[lo:hi]] = ind_s[lo:hi] - c * CPC
        # pool one-hot columns: poolT[p, s] = center-within-block of slot
        # s*128+p, or -1 for padding
        cl = ind_l.reshape(ntile, 128) - (
            (np.arange(ntile) // S_max) * 128)[:, None]
        poolT = np.where(ind_l.reshape(ntile, 128) >= 0, cl, -1).T.astype(
            np.float32).copy()

        oh9 = np.zeros((9, CPAD), np.float16)
        cn = np.arange(c * CPC, (c + 1) * CPC)
        oh9[x[cn, 0] * 3 + x[cn, 1], np.arange(CPC)] = 1.0

        per_core.append(dict(cnt19=cnt19, gidxT=gidxT, dstT=dstT,
                             poolT=poolT, oh9=oh9))
    meta = dict(S_max=S_max, SLOTS=SLOTS, T_E=T_E, ntile=ntile,
                tiles_per_nt=[int(v) for v in tiles_per_nt],
                npiece=NPIECE, piece=PIECE)
    return per_core, meta


def _weight_maps(inputs):
    """Per-core replicated weight/constant tensors (host casts only)."""
    f16 = np.float16
    f32 = np.float32
    emb1 = np.asarray(inputs["emb1"], f32)
    emb2 = np.asarray(inputs["emb2"], f32)
    ee1 = np.asarray(inputs["edge_e1"], f32)
    ee2 = np.asarray(inputs["edge_e2"], f32)
    W1 = np.asarray(inputs["W1"], f32)
    b1 = np.asarray(inputs["b1"], f32)
    W2 = np.asarray(inputs["W2"], f32)
    b2 = np.asarray(inputs["b2"], f32)

    # selection matrices (constants): TAB1[t] = emb1[t//3]+emb2[t%3] (t<9),
    # TAB1[9+u] = ee1[0][bond(u)] + ee2[0][dir(u)], u<9 real, u=9 selfloop.
    sel1t = np.zeros((120, 19), f16)
    sel2t = np.zeros((3, 19), f16)
    selbt = np.zeros((6, 19), f16)
    seldt = np.zeros((3, 19), f16)
    for t in range(9):
        sel1t[t // 3, t] = 1
        sel2t[t % 3, t] = 1
    for u in range(9):
        selbt[u // 3, 9 + u] = 1
        seldt[u % 3, 9 + u] = 1
    selbt[4, 18] = 1
    seldt[0, 18] = 1
    selbt2 = np.zeros((6, 10), f16)
    seldt2 = np.zeros((3, 10), f16)
    for u in range(9):
        selbt2[u // 3, u] = 1
        seldt2[u % 3, u] = 1
    selbt2[4, 9] = 1
    seldt2[0, 9] = 1

    return dict(
        emb1f=emb1.astype(f16), emb2f=emb2.astype(f16),
        ee1a=ee1[0].astype(f16), ee1b=ee1[1].astype(f16),
        ee2a=ee2[0].astype(f16), ee2b=ee2[1].astype(f16),
        sel1t=sel1t, sel2t=sel2t, selbt=selbt, seldt=seldt,
        selbt2=selbt2, seldt2=seldt2,
        w1=W1.astype(f16), w2=W2.astype(f16),
        b1t=b1.reshape(2, 4, 128, 1).astype(f32),
        b2f=b2.reshape(2, 1, 256).astype(f16),
        wp=np.asarray(inputs["Wp"], f32).astype(f16),
        bpt=np.asarray(inputs["bp"], f32).reshape(2, 128, 1),
        bngt=np.asarray(inputs["bn_cat_g"], f32).reshape(4, 128, 1),
        bnbt=np.asarray(inputs["bn_cat_b"], f32).reshape(4, 128, 1),
        ngt=np.asarray(inputs["norm_g"], f32).reshape(2, 128, 1),
        nbt=np.asarray(inputs["norm_b"], f32).reshape(2, 128, 1),
    )


# ----------------------------------------------------------------------------
# bass kernel
# ----------------------------------------------------------------------------
def _build(meta):
    from concourse import bass, bacc, mybir, tile
    from concourse.masks import make_identity

    f16 = mybir.dt.float16
    f32 = mybir.dt.float32
    i32 = mybir.dt.int32
    AF = mybir.ActivationFunctionType
    OP = mybir.AluOpType

    SLOTS = meta["SLOTS"]
    T_E = meta["T_E"]
    NTILE = meta["ntile"]
    TPN = meta["tiles_per_nt"]
    S_max = meta["S_max"]
    NCH = NTILE // 4
    NPIECE = meta["npiece"]
    PIECE = meta["piece"]
    CH_PER_PIECE = NCH // NPIECE

    nc = bacc.Bacc("TRN2", target_bir_lowering=False, debug=False,
                   num_devices=NCORE)

    def din(name, shape, dt):
        return nc.dram_tensor(name, shape, dt, kind="ExternalInput")

    cnt19 = din("cnt19", [19, SLOTS], f16)
    gidxT = din("gidxT", [128, T_E], f32)
    dstT = din("dstT", [128, T_E], f32)
    poolT = din("poolT", [128, NTILE], f32)
    oh9 = din("oh9", [9, CPAD], f16)
    emb1f = din("emb1f", [120, 256], f16)
    emb2f = din("emb2f", [3, 256], f16)
    ee1a = din("ee1a", [6, 256], f16)
    ee1b = din("ee1b", [6, 256], f16)
    ee2a = din("ee2a", [3, 256], f16)
    ee2b = din("ee2b", [3, 256], f16)
    sel1t = din("sel1t", [120, 19], f16)
    sel2t = din("sel2t", [3, 19], f16)
    selbt = din("selbt", [6, 19], f16)
    seldt = din("seldt", [3, 19], f16)
    selbt2 = din("selbt2", [6, 10], f16)
    seldt2 = din("seldt2", [3, 10], f16)
    w1 = din("w1", [2, 256, 512], f16)
    w2 = din("w2", [2, 512, 256], f16)
    b1t = din("b1t", [2, 4, 128, 1], f32)
    b2f = din("b2f", [2, 1, 256], f16)
    wp = din("wp", [512, 256], f16)
    bpt = din("bpt", [2, 128, 1], f32)
    bngt = din("bngt", [4, 128, 1], f32)
    bnbt = din("bnbt", [4, 128, 1], f32)
    ngt = din("ngt", [2, 128, 1], f32)
    nbt = din("nbt", [2, 128, 1], f32)
    out = nc.dram_tensor("out", [CPAD, 256], f32, kind="ExternalOutput")

    with tile.TileContext(nc) as tc:
        with (
            tc.tile_pool(name="const", bufs=1) as cpool,
            tc.tile_pool(name="wide", bufs=1) as wide,
            tc.tile_pool(name="work", bufs=3) as work,
            tc.tile_pool(name="mids", bufs=8) as midp,
            tc.tile_pool(name="aggp", bufs=6) as aggp,
            tc.tile_pool(name="statp", bufs=6) as statp,
            tc.tile_pool(name="msgs", bufs=24) as msgp,
            tc.tile_pool(name="ohs", bufs=16) as ohp,
            tc.tile_pool(name="h2s", bufs=14) as h2p,
            tc.tile_pool(name="ps512", bufs=2, space="PSUM") as ps512,
            tc.tile_pool(name="ps256", bufs=2, space="PSUM") as ps256,
            tc.tile_pool(name="ps128", bufs=2, space="PSUM") as ps128,
            tc.tile_pool(name="dram", bufs=1, space="DRAM") as dram,
        ):
            # ---------------- constants / weights into SBUF ----------------
            _ldc = [0]

            def load(pool, src, shape, dt):
                _ldc[0] += 1
                nm = f"ld{_ldc[0]}"
                t = pool.tile(shape, dt, name=nm, tag=nm)
                nc.sync.dma_start(out=t[:], in_=src)
                return t

            sel1_sb = load(cpool, sel1t[:, :], [120, 19], f16)
            sel2_sb = load(cpool, sel2t[:, :], [3, 19], f16)
            selb_sb = load(cpool, selbt[:, :], [6, 19], f16)
            seld_sb = load(cpool, seldt[:, :], [3, 19], f16)
            selb2_sb = load(cpool, selbt2[:, :], [6, 10], f16)
            seld2_sb = load(cpool, seldt2[:, :], [3, 10], f16)
            emb1_sb = load(cpool, emb1f[:, :], [120, 256], f16)
            emb2_sb = load(cpool, emb2f[:, :], [3, 256], f16)
            ee1a_sb = load(cpool, ee1a[:, :], [6, 256], f16)
            ee1b_sb = load(cpool, ee1b[:, :], [6, 256], f16)
            ee2a_sb = load(cpool, ee2a[:, :], [3, 256], f16)
            ee2b_sb = load(cpool, ee2b[:, :], [3, 256], f16)
            w1_sb = [[load(cpool, w1[l, k * 128:(k + 1) * 128, :],
                           [128, 512], f16) for k in range(2)]
                     for l in range(2)]
            w2_sb = [[load(cpool, w2[l, k * 128:(k + 1) * 128, :],
                           [128, 256], f16) for k in range(4)]
                     for l in range(2)]
            wp_sb = [load(cpool, wp[k * 128:(k + 1) * 128, :],
                          [128, 256], f16) for k in range(4)]
            b1_sb = [[load(cpool, b1t[l, m], [128, 1], f32) for m in range(4)]
                     for l in range(2)]
            b2_sb = [load(cpool, b2f[l], [1, 256], f16) for l in range(2)]
            bp_sb = [load(cpool, bpt[c2], [128, 1], f32) for c2 in range(2)]
            bng_sb = [load(cpool, bngt[t], [128, 1], f32) for t in range(4)]
            bnb_sb = [load(cpool, bnbt[t], [128, 1], f32) for t in range(4)]
            ng_sb = [load(cpool, ngt[t], [128, 1], f32) for t in range(2)]
            nb_sb = [load(cpool, nbt[t], [128, 1], f32) for t in range(2)]
            oh9_sb = load(cpool, oh9[:, :], [9, CPAD], f16)
            gidxT_sb = load(cpool, gidxT[:, :], [128, T_E], f32)
            dstT_sb = load(cpool, dstT[:, :], [128, T_E], f32)
            poolT_sb = load(cpool, poolT[:, :], [128, NTILE], f32)

            ones_sb = cpool.tile([1, 128], f16)
            nc.vector.memset(ones_sb[:], 1.0)
            eps_sb = cpool.tile([128, 1], f32)
            nc.vector.memset(eps_sb[:], EPS)
            ident = cpool.tile([128, 128], f32)
            make_identity(nc, ident[:])
            ident16 = cpool.tile([128, 128], f16)
            make_identity(nc, ident16[:])
            iota_sb = cpool.tile([128, 128], f32)
            nc.gpsimd.iota(iota_sb[:], pattern=[[1, 128]], base=0,
                           channel_multiplier=0,
                           allow_small_or_imprecise_dtypes=True)

            # tables: TAB1 [19, 256] = sel1t^T@emb1 + sel2t^T@emb2 (+edge l0)
            tab_ps = ps256.tile([19, 256], f32, space="PSUM", tag="ps256")
            nc.tensor.matmul(tab_ps[:], lhsT=sel1_sb[:], rhs=emb1_sb[:],
                             start=True, stop=False)
            nc.tensor.matmul(tab_ps[:], lhsT=sel2_sb[:], rhs=emb2_sb[:],
                             start=False, stop=False)
            nc.tensor.matmul(tab_ps[:], lhsT=selb_sb[:], rhs=ee1a_sb[:],
                             start=False, stop=False)
            nc.tensor.matmul(tab_ps[:], lhsT=seld_sb[:], rhs=ee2a_sb[:],
                             start=False, stop=True)
            tab1_sb = cpool.tile([19, 256], f16)
            nc.vector.tensor_copy(out=tab1_sb[:], in_=tab_ps[:])

            tab2_ps = ps256.tile([10, 256], f32, space="PSUM", tag="ps256")
            nc.tensor.matmul(tab2_ps[:], lhsT=selb2_sb[:], rhs=ee1b_sb[:],
                             start=True, stop=False)
            nc.tensor.matmul(tab2_ps[:], lhsT=seld2_sb[:], rhs=ee2b_sb[:],
                             start=False, stop=True)
            ee2_sb = cpool.tile([10, 256], f16)
            nc.vector.tensor_copy(out=ee2_sb[:], in_=tab2_ps[:])

            # DRAM bounces
            h1loc = dram.tile([SLOTS, 256], f16)
            h1full = dram.tile([NCORE * SLOTS, 256], f16, addr_space="Shared")
            st1loc = dram.tile([512, 2], f32)
            st1glob = dram.tile([512, 2], f32)
            st2loc = dram.tile([256, 2], f32)
            st2glob = dram.tile([256, 2], f32)
            barloc = dram.tile([128, 2], f32)
            barglob = dram.tile([128, 2], f32)

            # ------------- origin half of cat (overlaps layer 1) -----------
            cat_sb = [wide.tile([128, CPAD], f16, tag=f"cat{t}",
                                name=f"cat{t}") for t in range(4)]
            for k in range(2):
                for w in range(CPAD // 512):
                    op_ = ps512.tile([128, 512], f32, space="PSUM",
                                     tag="ps512")
                    nc.tensor.matmul(
                        op_[:], lhsT=tab1_sb[0:9, k * 128:(k + 1) * 128],
                        rhs=oh9_sb[:, w * 512:(w + 1) * 512],
                        start=True, stop=True)
                    nc.vector.tensor_copy(
                        out=cat_sb[k][:, w * 512:(w + 1) * 512], in_=op_[:])

            # ---------------- shared MLP block (fm chunk of 512 rows) ------
            def mlp(l, agg_sb, h_store):
                """agg_sb: 2 x [128, 512] f16 fm. Calls h_store(r, psum) for
                4 row-tiles of 128 rows."""
                mid_sb = []
                for m in range(4):
                    mp = ps512.tile([128, 512], f32, space="PSUM", tag="ps512")
                    for k in range(2):
                        nc.tensor.matmul(
                            mp[:],
                            lhsT=w1_sb[l][k][:, m * 128:(m + 1) * 128],
                            rhs=agg_sb[k][:], start=(k == 0), stop=(k == 1))
                    ms = midp.tile([128, 512], f16, tag="mid")
                    if m % 2 == 0:
                        nc.scalar.activation(out=ms[:], in_=mp[:],
                                             func=AF.Relu,
                                             bias=b1_sb[l][m][:])
                    else:
                        # split relu+bias across scalar and vector engines
                        nc.vector.tensor_scalar(
                            out=ms[:], in0=mp[:],
                            scalar1=b1_sb[l][m][:, 0:1], scalar2=0.0,
                            op0=OP.add, op1=OP.max)
                    mid_sb.append(ms)
                for r in range(4):
                    hp = ps256.tile([128, 256], f32, space="PSUM", tag="ps256")
                    for k in range(4):
                        nc.tensor.matmul(
                            hp[:], lhsT=mid_sb[k][:, r * 128:(r + 1) * 128],
                            rhs=w2_sb[l][k][:],
                            start=(k == 0), stop=False)
                    nc.tensor.matmul(hp[:], lhsT=ones_sb[:], rhs=b2_sb[l][:],
                                     start=False, stop=True)
                    h_store(r, hp)

            def stats_push(tiles, n_real, loc, t_base, nt_):
                for t in range(nt_):
                    s_sb = statp.tile([128, 2], f32, tag="stat")
                    nc.vector.tensor_reduce(
                        out=s_sb[:, 0:1], in_=tiles[t][:, 0:n_real],
                        axis=mybir.AxisListType.X, op=OP.add)
                    sq = wide.tile([128, CPC], f32, tag="sqtmp")
                    nc.vector.tensor_tensor(
                        out=sq[:], in0=tiles[t][:, 0:n_real],
                        in1=tiles[t][:, 0:n_real], op=OP.mult)
                    nc.vector.tensor_reduce(
                        out=s_sb[:, 1:2], in_=sq[:],
                        axis=mybir.AxisListType.X, op=OP.add)
                    nc.sync.dma_start(
                        out=loc[(t_base + t) * 128:(t_base + t + 1) * 128, :],
                        in_=s_sb[:])

            def stats_pull(glob, t_base, nt_):
                outs = []
                for t in range(nt_):
                    g_sb = statp.tile([128, 2], f32, tag="gstat")
                    nc.sync.dma_start(
                        out=g_sb[:],
                        in_=glob[(t_base + t) * 128:(t_base + t + 1) * 128, :])
                    outs.append(g_sb)
                return outs

            # origin-half BN1 stats: reduce + AllReduce early, hidden
            # under layer 1
            stats_push(cat_sb, CPC, st1loc, 0, 2)
            nc.gpsimd.collective_compute(
                "AllReduce", OP.add,
                replica_groups=[list(range(NCORE))],
                ins=[st1loc[0:256, :].opt()], outs=[st1glob[0:256, :].opt()])

            # ---------------- phase B: layer 1 ----------------
            for ch in range(NCH):
                c0 = ch * 512
                cnt_sb = work.tile([19, 512], f16, tag="cnt")
                nc.sync.dma_start(out=cnt_sb[:],
                                  in_=cnt19[:, c0:c0 + 512])
                agg_sb = []
                for k in range(2):
                    ap_ = ps512.tile([128, 512], f32, space="PSUM",
                                     tag="ps512")
                    nc.tensor.matmul(ap_[:],
                                     lhsT=tab1_sb[:, k * 128:(k + 1) * 128],
                                     rhs=cnt_sb[:], start=True, stop=True)
                    asb = aggp.tile([128, 512], f16, tag="agg")
                    nc.vector.tensor_copy(out=asb[:], in_=ap_[:])
                    agg_sb.append(asb)

                h1w = work.tile([128, 1024], f16, tag="h1w")

                def store_h1(rt, hp, h1w=h1w):
                    nc.scalar.activation(
                        out=h1w[:, rt * 256:(rt + 1) * 256], in_=hp[:],
                        func=AF.Relu)

                mlp(0, agg_sb, store_h1)
                nc.sync.dma_start(out=h1loc[c0:c0 + 512, :], in_=h1w[:])

            nc.gpsimd.collective_compute(
                "AllGather", OP.bypass,
                replica_groups=[list(range(NCORE))],
                ins=[h1loc[:].opt()], outs=[h1full[:].opt()])
            gidxT_i = cpool.tile([128, T_E], i32)
            nc.vector.tensor_copy(out=gidxT_i[:], in_=gidxT_sb[:])

            # ------------- phase D+E: layer 2 + interleaved pooling --------
            h2_tiles = [None] * NTILE
            pooled = [0]

            def onehot(col_ap):
                oh = ohp.tile([128, 128], f16, tag="oh")
                nc.vector.tensor_scalar(
                    out=oh[:], in0=iota_sb[:], scalar1=col_ap,
                    scalar2=None, op0=OP.is_equal)
                return oh

            def pool_block(b):
                pps = [ps128.tile([128, 128], f32, space="PSUM",
                                  tag=f"g{k}", name=f"pps{k}")
                       for k in range(2)]
                for s in range(S_max):
                    nt = b * S_max + s
                    oh = onehot(poolT_sb[:, nt:nt + 1])
                    for k in range(2):
                        nc.tensor.matmul(
                            pps[k][:],
                            lhsT=h2_tiles[nt][:, k * 128:(k + 1) * 128],
                            rhs=oh[:], start=(s == 0),
                            stop=(s == S_max - 1))
                for k in range(2):
                    nc.vector.tensor_copy(
                        out=cat_sb[2 + k][:, b * 128:(b + 1) * 128],
                        in_=pps[k][:])

            t0 = 0
            for ch in range(NCH):
                c0 = ch * 512
                # own h1 rows (flat layout matches the layer-1 store)
                h1own = work.tile([128, 1024], f16, tag="h1own")
                nc.sync.dma_start(out=h1own[:], in_=h1loc[c0:c0 + 512, :])
                cntE_sb = work.tile([10, 512], f16, tag="cntE")
                nc.sync.dma_start(out=cntE_sb[:],
                                  in_=cnt19[9:19, c0:c0 + 512])
                agg_sb = [aggp.tile([128, 512], f16, tag="agg2",
                                    name=f"agg2sb{_k}") for _k in range(2)]
                tb = t0
                for j in range(4):
                    nt = ch * 4 + j
                    gps = [ps128.tile([128, 128], f32, space="PSUM",
                                      tag=f"g{k}", name=f"gps{k}")
                           for k in range(2)]
                    for k in range(2):
                        nc.tensor.matmul(
                            gps[k][:], lhsT=ee2_sb[:, k * 128:(k + 1) * 128],
                            rhs=cntE_sb[:, j * 128:(j + 1) * 128],
                            start=True, stop=False)
                        # self-loop: agg[:, s] += h1own[s, :]^T
                        nc.tensor.matmul(
                            gps[k][:],
                            lhsT=h1own[:, j * 256 + k * 128:
                                       j * 256 + (k + 1) * 128],
                            rhs=ident16[:], start=False, stop=(TPN[nt] == 0))
                    for t in range(tb, tb + TPN[nt]):
                        msg = msgp.tile([128, 256], f16, tag="msg")
                        nc.gpsimd.indirect_dma_start(
                            out=msg[:], out_offset=None,
                            in_=h1full[:],
                            in_offset=bass.IndirectOffsetOnAxis(
                                ap=gidxT_i[:, t:t + 1], axis=0))
                        oh = onehot(dstT_sb[:, t:t + 1])
                        last = t == tb + TPN[nt] - 1
                        for k in range(2):
                            nc.tensor.matmul(
                                gps[k][:],
                                lhsT=msg[:, k * 128:(k + 1) * 128],
                                rhs=oh[:], start=False, stop=last)
                    tb += TPN[nt]
                    for k in range(2):
                        nc.vector.tensor_copy(
                            out=agg_sb[k][:, j * 128:(j + 1) * 128],
                            in_=gps[k][:])
                t0 = tb

                def store_h2(rt, hp, ch=ch):
                    hs = h2p.tile([128, 256], f16, tag="h2")
                    nc.scalar.activation(out=hs[:], in_=hp[:], func=AF.Relu)
                    h2_tiles[ch * 4 + rt] = hs

                mlp(1, agg_sb, store_h2)

                bdone = ((ch + 1) * 4) // S_max
                for b in range(pooled[0], bdone):
                    pool_block(b)
                pooled[0] = bdone
            assert t0 == T_E
            for b in range(pooled[0], NBLK):
                pool_block(b)

            # ---------------- phase F: BN1 -> proj -> BN2 -> out -----------
            def stats(tiles, n_real, loc, glob, nt_):
                stats_push(tiles, n_real, loc, 0, nt_)
                nc.gpsimd.collective_compute(
                    "AllReduce", OP.add,
                    replica_groups=[list(range(NCORE))],
                    ins=[loc[:].opt()], outs=[glob[:].opt()])
                return stats_pull(glob, 0, nt_)

            def scale_bias(g_sb, gam, bet, sfx):
                # mu = s0/N; var = s1/N - mu^2; rstd = 1/sqrt(var+eps)
                mu = work.tile([128, 1], f32, tag=f"mu{sfx}", name="mu")
                nc.vector.tensor_scalar_mul(mu[:], g_sb[:, 0:1], 1.0 / N)
                var = work.tile([128, 1], f32, tag=f"var{sfx}", name="var")
                nc.vector.tensor_scalar_mul(var[:], g_sb[:, 1:2], 1.0 / N)
                musq = work.tile([128, 1], f32, tag=f"musq{sfx}", name="musq")
                nc.vector.tensor_tensor(out=musq[:], in0=mu[:], in1=mu[:],
                                        op=OP.mult)
                nc.vector.tensor_tensor(out=var[:], in0=var[:], in1=musq[:],
                                        op=OP.subtract)
                sd = work.tile([128, 1], f32, tag=f"sd{sfx}", name="sd")
                nc.scalar.activation(out=sd[:], in_=var[:], func=AF.Sqrt,
                                     bias=eps_sb[:, 0:1])
                rstd = work.tile([128, 1], f32, tag=f"rstd{sfx}", name="rstd")
                nc.vector.reciprocal(rstd[:], sd[:])
                sc = work.tile([128, 1], f32, tag=f"sc{sfx}", name="sc")
                nc.vector.tensor_tensor(out=sc[:], in0=rstd[:], in1=gam[:],
                                        op=OP.mult)
                bi = work.tile([128, 1], f32, tag=f"bi{sfx}", name="bi")
                nc.vector.tensor_tensor(out=bi[:], in0=mu[:], in1=sc[:],
                                        op=OP.mult)
                nc.vector.tensor_tensor(out=bi[:], in0=bet[:], in1=bi[:],
                                        op=OP.subtract)
                return sc, bi

            stats_push(cat_sb[2:], CPC, st1loc, 2, 2)
            nc.gpsimd.collective_compute(
                "AllReduce", OP.add,
                replica_groups=[list(range(NCORE))],
                ins=[st1loc[256:512, :].opt()],
                outs=[st1glob[256:512, :].opt()])
            g1 = stats_pull(st1glob, 0, 4)
            # fold BN1 into the projection: out2 = sum_k cat_k @ (sc_k*wp_k)
            #   + [sum_k bi_k @ wp_k + bp]
            bi16s = []
            for t in range(4):
                sc, bi = scale_bias(g1[t], bng_sb[t], bnb_sb[t], f"a{t}")
                bi16 = statp.tile([128, 1], f16, tag="bi16")
                nc.vector.tensor_copy(out=bi16[:], in_=bi[:])
                bi16s.append((sc, bi16))
            bias_sb = []
            for c2 in range(2):
                bps_ = ps128.tile([128, 1], f32, space="PSUM", tag="g0",
                                  name="bps_")
                for k in range(4):
                    nc.tensor.matmul(
                        bps_[:], lhsT=wp_sb[k][:, c2 * 128:(c2 + 1) * 128],
                        rhs=bi16s[k][1][:], start=(k == 0), stop=(k == 3))
                bb = statp.tile([128, 1], f32, tag="bb", name="bb")
                nc.vector.tensor_tensor(out=bb[:], in0=bps_[:],
                                        in1=bp_sb[c2][:], op=OP.add)
                bias_sb.append(bb)
            for k in range(4):
                nc.vector.tensor_scalar(
                    out=wp_sb[k][:], in0=wp_sb[k][:],
                    scalar1=bi16s[k][0][:, 0:1], scalar2=None, op0=OP.mult)

            out2_sb = [wide.tile([128, CPAD], f32, tag=f"o2_{c2}",
                                 name=f"o2sb{c2}") for c2 in range(2)]
            for w in range(CPAD // 512):
                for c2 in range(2):
                    pp = ps512.tile([128, 512], f32, space="PSUM",
                                     tag="ps512")
                    for k in range(4):
                        nc.tensor.matmul(
                            pp[:],
                            lhsT=wp_sb[k][:, c2 * 128:(c2 + 1) * 128],
                            rhs=cat_sb[k][:, w * 512:(w + 1) * 512],
                            start=(k == 0), stop=(k == 3))
                    nc.vector.tensor_scalar(
                        out=out2_sb[c2][:, w * 512:(w + 1) * 512], in0=pp[:],
                        scalar1=bias_sb[c2][:, 0:1], scalar2=None, op0=OP.add)

            g2 = stats(out2_sb, CPC, st2loc, st2glob, 2)
            for c2 in range(2):
                sc, bi = scale_bias(g2[c2], ng_sb[c2], nb_sb[c2], f"b{c2}")
                nc.vector.tensor_scalar(
                    out=out2_sb[c2][:], in0=out2_sb[c2][:], scalar1=sc[:, 0:1],
                    scalar2=bi[:, 0:1], op0=OP.mult, op1=OP.add)

            for w0 in range(0, NBLK, 4):
                os_ = work.tile([128, 1024], f32, tag="outrm")
                for wb in range(4):
                    for c2 in range(2):
                        tp = ps128.tile([128, 128], f32, space="PSUM",
                                        tag="g0")
                        nc.tensor.transpose(
                            out=tp[:],
                            in_=out2_sb[c2][:, (w0 + wb) * 128:
                                            (w0 + wb + 1) * 128],
                            identity=ident[:])
                        nc.vector.tensor_copy(
                            out=os_[:, wb * 256 + c2 * 128:
                                    wb * 256 + (c2 + 1) * 128], in_=tp[:])
                _ob = out[0:1, :]
                out_ap = bass.AP(_ob.tensor, w0 * 128 * 256,
                                 [[256, 128], [128 * 256, 4], [1, 256]])
                nc.sync.dma_start(out=out_ap, in_=os_[:])

    nc.compile()
    return nc


_CACHE = {}


def kernel(**inputs):
    from concourse.bass_utils import run_bass_kernel_spmd

    per_core, meta = _preprocess(inputs)
    wm = _weight_maps(inputs)

    key = (meta["SLOTS"], meta["T_E"], tuple(meta["tiles_per_nt"]))
    if key not in _CACHE:
        _CACHE[key] = _build(meta)
    nc = _CACHE[key]

    in_maps = []
    for c in range(NCORE):
        m = dict(per_core[c])
        m.update(wm)
        in_maps.append(m)

    trace = bool(int(os.environ.get("KERNEL_TRACE", "0")))
    res = run_bass_kernel_spmd(nc, in_maps, list(range(NCORE)), trace=trace)
    kernel.last_results = res

    outs = [res.results[c]["out"][:CPC] for c in range(NCORE)]
    return np.concatenate(outs, 0).astype(np.float32)
